# revision 2
# baseline (speedup 1.0000x reference)
"""CryoProjector Trainium2 kernel.

Math: clean[b,i,j] = sum_n exp(-((i-py_n)^2 + (j-px_n)^2) / (2*sigma^2))
The Gaussian is separable, so with
    Gy[n,i] = exp(-(i-py_n)^2 / (2 s^2)),  Gx[n,j] = exp(-(j-px_n)^2 / (2 s^2))
clean[b] = Gy^T @ Gx  -- a (H x N) @ (N x W) matmul done on the TensorEngine.

Sharding: 8 cores = (batch b in 0..3) x (row-half h in 0..1). Each core
computes a (64,128) slice of clean/noisy for its batch. Row offset 64h is
folded into py so the device program is identical across cores (pure SPMD).

Point->partition layout: x[b] is DMA'd contiguously as (128, 48); partition p
holds points 16p..16p+15 (3 coords interleaved). Matmul K-chunk t covers
points {16p + t : p in 0..127}, so the per-chunk per-partition scalar
px/py[:, t] feeds broadcast-AP tensor ops directly -- no transpose needed.
The chunk permutation cancels in the matmul sum.
"""

import numpy as np

H = W = 128
B, N = 4, 2048
SCALE = min(H, W) / 2.0 * 0.8  # 51.2
ATOM_SIGMA = 1.5
INV2SIG2 = 1.0 / (2.0 * ATOM_SIGMA**2)
SIGMA_NOISE = 0.1

_NCORES = 8
_CHUNKS = 16  # N / 128

_cache = {}


def _build_nc():
    import concourse.bacc as bacc
    import concourse.mybir as mybir
    from concourse import tile

    f32 = mybir.dt.float32
    f16 = mybir.dt.float16
    i32 = mybir.dt.int32
    AF = mybir.ActivationFunctionType
    OP = mybir.AluOpType

    nc = bacc.Bacc("TRN2", target_bir_lowering=False, debug=False,
                   num_devices=_NCORES)

    # packed input: cols 0:48 = x[b] (128,48); 48:51 rot row0; 51 cx;
    # 52:55 rot row1; 55 cy_eff = 64 - 64h
    PK = nc.declare_dram_parameter("packed", (128, 56), f32, isOutput=False)
    NZ = nc.declare_dram_parameter("noise", (64, 128), f32, isOutput=False)
    OUT = nc.declare_dram_parameter("out", (64, 256), f32, isOutput=True)

    with tile.TileContext(nc) as tc:
        with (
            tc.tile_pool(name="p", bufs=1) as P,
            tc.tile_pool(name="ps", bufs=1, space="PSUM") as PS,
        ):
            pk = P.tile([128, 56], f32)
            nc.sync.dma_start(pk[:], PK[:])
            nz = P.tile([64, 128], f32)
            nc.sync.dma_start(nz[:], NZ[:])

            # Warm the exp table on ACT early (overlaps DMA/DVE head).
            w0 = P.tile([128, 1], f32)
            nc.vector.memset(w0[:], 0.0)
            w1 = P.tile([128, 1], f32)
            nc.scalar.activation(w1[:], w0[:], AF.Exp)

            # Pixel grid 0..127 generated on the (otherwise idle) Pool engine.
            gi = P.tile([128, 128], i32)
            nc.gpsimd.iota(gi[:], pattern=[[1, 128]], base=0,
                           channel_multiplier=0)
            g = P.tile([128, 128], f32)
            nc.vector.tensor_copy(g[:], gi[:])

            # px, py: (128,16), px[p,t] = SCALE*(rot0 . x_{16p+t}) + 64
            pkv = pk[:]
            x3 = pkv[:, 0:48].rearrange("p (t c) -> p c t", c=3)

            def project(c0, coff):
                t0 = P.tile([128, 16], f32, tag=f"t0{coff}")
                nc.vector.tensor_scalar_mul(t0[:], x3[:, 0, :],
                                            pkv[:, c0:c0 + 1])
                t1 = P.tile([128, 16], f32, tag=f"t1{coff}")
                nc.vector.scalar_tensor_tensor(
                    t1[:], x3[:, 1, :], pkv[:, c0 + 1:c0 + 2], t0[:],
                    OP.mult, OP.add)
                r = P.tile([128, 16], f32, tag=f"r{coff}")
                nc.vector.scalar_tensor_tensor(
                    r[:], x3[:, 2, :], pkv[:, c0 + 2:c0 + 3], t1[:],
                    OP.mult, OP.add)
                o = P.tile([128, 16], f32, tag=f"o{coff}")
                nc.vector.tensor_scalar(o[:], r[:], SCALE,
                                        pkv[:, coff:coff + 1],
                                        OP.mult, OP.add)
                return o

            px = project(48, 51)
            py = project(52, 55)

            # Big broadcast-AP ops: dy[p, t, i] = g[i] - py[p, t]
            def gauss(pt, cols, name):
                nchunk = _CHUNKS
                d = P.tile([128, nchunk * cols], f32, tag=f"d{name}")
                d3 = d[:].rearrange("p (k j) -> p k j", j=cols)
                gb = g[:][:, 0:cols].unsqueeze(1).broadcast_to(
                    [128, nchunk, cols])
                pb = pt[:].unsqueeze(2).broadcast_to([128, nchunk, cols])
                nc.vector.tensor_tensor(d3, gb, pb, OP.subtract)
                sq = P.tile([128, nchunk * cols], f32, tag=f"sq{name}")
                nc.vector.tensor_mul(sq[:], d[:], d[:])
                e = P.tile([128, nchunk * cols], f16, tag=f"e{name}")
                nc.scalar.activation(e[:], sq[:], AF.Exp, scale=-INV2SIG2)
                return e

            ey = gauss(py, 64, "y")
            ex = gauss(px, 128, "x")

            # 16 accumulating matmuls: psum[i,j] += ey_k^T @ ex_k
            pst = PS.tile([64, 128], f32)
            eyv = ey[:].rearrange("p (k j) -> p k j", j=64)
            exv = ex[:].rearrange("p (k j) -> p k j", j=128)
            for k in range(_CHUNKS):
                nc.tensor.matmul(pst[:], eyv[:, k, :], exv[:, k, :],
                                 start=(k == 0), stop=(k == _CHUNKS - 1))

            # Tail: out[:, :128] = noisy = clean + 0.1*noise; out[:,128:] = clean
            ob = P.tile([64, 256], f32)
            nc.vector.tensor_copy(ob[:][:, 128:256], pst[:])
            nc.vector.scalar_tensor_tensor(ob[:][:, 0:128], nz[:],
                                           SIGMA_NOISE, pst[:],
                                           OP.mult, OP.add)
            nc.sync.dma_start(OUT[:], ob[:])

    nc.compile()
    return nc


def _get_nc():
    if "nc" not in _cache:
        _cache["nc"] = _build_nc()
    return _cache["nc"]


def make_in_maps(x, rot, noise):
    in_maps = []
    for c in range(_NCORES):
        b, h = c // 2, c % 2
        pkd = np.empty((128, 56), np.float32)
        pkd[:, :48] = np.ascontiguousarray(x[b]).reshape(128, 48)
        pkd[:, 48:51] = rot[b, 0]
        pkd[:, 51] = W / 2.0
        pkd[:, 52:55] = rot[b, 1]
        pkd[:, 55] = H / 2.0 - 64.0 * h
        in_maps.append({
            "packed": pkd,
            "noise": np.ascontiguousarray(noise[b, 64 * h:64 * h + 64, :]),
        })
    return in_maps


def assemble(results):
    noisy = np.empty((B, H, W), np.float32)
    clean = np.empty((B, H, W), np.float32)
    for c in range(_NCORES):
        b, h = c // 2, c % 2
        out = results[c]["out"]
        noisy[b, 64 * h:64 * h + 64, :] = out[:, :128]
        clean[b, 64 * h:64 * h + 64, :] = out[:, 128:]
    return noisy, clean


def kernel(x, rot_matrices, noise):
    from concourse.bass_utils import run_bass_kernel_spmd

    x = np.asarray(x, dtype=np.float32)
    rot = np.asarray(rot_matrices, dtype=np.float32)
    noise = np.asarray(noise, dtype=np.float32)

    nc = _get_nc()
    res = run_bass_kernel_spmd(nc, make_in_maps(x, rot, noise),
                               list(range(_NCORES)))
    noisy, clean = assemble(res.results)
    return noisy, rot, clean


# revision 10
# speedup vs baseline: 1.0264x; 1.0264x over previous
"""CryoProjector Trainium2 kernel.

Math: clean[b,i,j] = sum_n exp(-((i-py_n)^2 + (j-px_n)^2) / (2*sigma^2))
The Gaussian is separable, so with
    Gy[n,i] = exp(-(i-py_n)^2 / (2 s^2)),  Gx[n,j] = exp(-(j-px_n)^2 / (2 s^2))
clean[b] = Gy^T @ Gx  -- a (H x N) @ (N x W) matmul done on the TensorEngine.

Sharding: 8 cores = (batch b in 0..3) x (row-half h in 0..1). Each core
computes a (64,128) slice of clean/noisy for its batch. Row offset 64h is
folded into py so the device program is identical across cores (pure SPMD).

Point->partition layout: x[b] is DMA'd contiguously as (128, 48); partition p
holds points 16p..16p+15 (3 coords interleaved). Matmul K-chunk t covers
points {16p + t : p in 0..127}, so the per-chunk per-partition scalar
px/py[:, t] feeds broadcast-AP tensor ops directly -- no transpose needed.
The chunk permutation cancels in the matmul sum.

Engine split (from profile): DVE builds dx and dx^2 (squares in fp16 for the
2x DVE mode), GpSimd builds dy/dy^2 (otherwise idle), ACT does the exps
(fp16 out), PE runs 16 accumulating fp16 matmuls plus warm-up matmuls to
raise its DVFS state before the real ones.
"""

import numpy as np

H = W = 128
B, N = 4, 2048
SCALE = min(H, W) / 2.0 * 0.8  # 51.2
ATOM_SIGMA = 1.5
INV2SIG2 = 1.0 / (2.0 * ATOM_SIGMA**2)
# Everything is pre-scaled by s = sqrt(1/(2 sigma^2)) so the squared
# differences are the exp argument directly and stay well inside fp16 range.
SQ_S = float(np.sqrt(INV2SIG2))
SIGMA_NOISE = 0.1

_NCORES = 8
_CHUNKS = 16  # N / 128

_cache = {}


def _build_nc():
    import concourse.bacc as bacc
    import concourse.mybir as mybir
    from concourse import tile

    f32 = mybir.dt.float32
    f16 = mybir.dt.float16
    AF = mybir.ActivationFunctionType
    OP = mybir.AluOpType
    AX = mybir.AxisListType

    nc = bacc.Bacc("TRN2", target_bir_lowering=False, debug=False,
                   num_devices=_NCORES)

    # packed input: cols 0:48 = x[b] (128,48); 48:51 rot row0; 51 cx;
    # 52:55 rot row1; 55 cy_eff = 64 - 64h
    PK = nc.declare_dram_parameter("packed", (128, 56), f32, isOutput=False)
    NZ = nc.declare_dram_parameter("noise", (64, 128), f32, isOutput=False)
    OUT = nc.declare_dram_parameter("out", (64, 256), f32, isOutput=True)

    with tile.TileContext(nc) as tc:
        with (
            tc.tile_pool(name="p", bufs=1) as P,
            tc.tile_pool(name="ps", bufs=1, space="PSUM") as PS,
        ):
            pk = P.tile([128, 56], f32)
            nc.sync.dma_start(pk[:], PK[:])
            nz = P.tile([64, 128], f32)
            nc.sync.dma_start(nz[:], NZ[:])

            # Warm the exp table on ACT early (overlaps DMA/DVE head).
            w0 = P.tile([128, 1], f32)
            nc.vector.memset(w0[:], 0.0)
            w1 = P.tile([128, 1], f32)
            nc.scalar.activation(w1[:], w0[:], AF.Exp)

            # Pixel grid 0..127 directly in fp32 (exact for ints < 2^24),
            # generated on the Pool engine which also does the y-side.
            g = P.tile([128, 128], f32)
            nc.gpsimd.iota(g[:], pattern=[[1, 128]], base=0,
                           channel_multiplier=0,
                           allow_small_or_imprecise_dtypes=True)
            gs = P.tile([128, 128], f32)
            nc.gpsimd.tensor_scalar_mul(gs[:], g[:], SQ_S)

            # PE warm-up fodder: fp16 zero tile + separate psum bank.
            wm = P.tile([128, 128], f16)
            nc.vector.memset(wm[:], 0.0)
            wps = PS.tile([64, 128], f32)

            def warm_mm(n=1):
                for _ in range(n):
                    nc.tensor.matmul(wps[:], wm[:, 0:64], wm[:],
                                     start=True, stop=True)

            warm_mm(3)

            # px, py: (128,16) each, pre-scaled by SQ_S.
            # prod[p, t, a, c] = x[16p+t, c] * rot[a, c]
            pkv = pk[:]
            xx = pkv[:, 0:48].rearrange("p (t c) -> p t c", c=3)
            x2 = xx.unsqueeze(2).broadcast_to([128, 16, 2, 3])
            rc = pkv[:, 48:54].rearrange("p (a c) -> p a c", c=3)
            r2 = rc.unsqueeze(1).broadcast_to([128, 16, 2, 3])
            prod = P.tile([128, 96], f32)
            nc.vector.tensor_tensor(
                prod[:].rearrange("p (t a c) -> p t a c", a=2, c=3),
                x2, r2, OP.mult)
            xr = P.tile([128, 32], f32)  # [p, (t, axis)]
            nc.vector.tensor_reduce(
                xr[:].rearrange("p (t a) -> p t a", a=2),
                prod[:].rearrange("p (t a c) -> p t a c", a=2, c=3),
                AX.X, OP.add)
            xr3 = xr[:].rearrange("p (t a) -> p t a", a=2)
            px = P.tile([128, 16], f32)
            nc.vector.tensor_scalar(px[:], xr3[:, :, 0], SCALE * SQ_S,
                                    pkv[:, 54:55], OP.mult, OP.add)
            py = P.tile([128, 16], f32)
            nc.vector.tensor_scalar(py[:], xr3[:, :, 1], SCALE * SQ_S,
                                    pkv[:, 55:56], OP.mult, OP.add)

            # ---- y side on GpSimd (Pool): dy = g*s - py_s, dy^2 -> fp16
            dy = P.tile([128, 16 * 64], f32)
            gy = gs[:][:, 0:64].unsqueeze(1).broadcast_to([128, 16, 64])
            pyb = py[:].unsqueeze(2).broadcast_to([128, 16, 64])
            nc.gpsimd.tensor_tensor(
                dy[:].rearrange("p (k j) -> p k j", j=64), gy, pyb,
                OP.subtract)
            sqy = P.tile([128, 16 * 64], f16)
            nc.gpsimd.tensor_tensor(sqy[:], dy[:], dy[:], OP.mult)

            warm_mm(2)

            # ---- x side on DVE, split in halves for ACT/PE overlap
            NSPLIT = 2
            KH = _CHUNKS // NSPLIT
            dx = P.tile([128, 16 * 128], f32)
            sqx = P.tile([128, 16 * 128], f16)
            gx = gs[:].unsqueeze(1).broadcast_to([128, KH, 128])
            dx3 = dx[:].rearrange("p (k j) -> p k j", j=128)
            pxb3 = px[:].unsqueeze(2).broadcast_to([128, 16, 128])
            for s in range(NSPLIT):
                ksl = slice(s * KH, (s + 1) * KH)
                nc.vector.tensor_tensor(dx3[:, ksl, :], gx,
                                        pxb3[:, ksl, :], OP.subtract)
                nc.vector.tensor_tensor(sqx[:][:, s * KH * 128:
                                               (s + 1) * KH * 128],
                                        dx[:][:, s * KH * 128:
                                              (s + 1) * KH * 128],
                                        dx[:][:, s * KH * 128:
                                              (s + 1) * KH * 128],
                                        OP.mult)

            # ---- exps on ACT (fp16 out); squares are the exp arg directly
            ey = P.tile([128, 16 * 64], f16)
            nc.scalar.activation(ey[:], sqy[:], AF.Exp, scale=-1.0)
            ex = P.tile([128, 16 * 128], f16)
            for s in range(NSPLIT):
                lo, hi = s * KH * 128, (s + 1) * KH * 128
                nc.scalar.activation(ex[:][:, lo:hi], sqx[:][:, lo:hi],
                                     AF.Exp, scale=-1.0)

            warm_mm(3)

            # ---- 16 accumulating matmuls
            pst = PS.tile([64, 128], f32)
            eyv = ey[:].rearrange("p (k j) -> p k j", j=64)
            exv = ex[:].rearrange("p (k j) -> p k j", j=128)
            for k in range(_CHUNKS):
                nc.tensor.matmul(pst[:], eyv[:, k, :], exv[:, k, :],
                                 start=(k == 0), stop=(k == _CHUNKS - 1))

            # ---- tail: noisy = clean + 0.1*noise on DVE; clean DMA'd
            # straight out of PSUM.
            nb = P.tile([64, 128], f32)
            nc.vector.scalar_tensor_tensor(nb[:], nz[:], SIGMA_NOISE,
                                           pst[:], OP.mult, OP.add)
            cb = P.tile([64, 128], f32)
            nc.scalar.activation(cb[:], pst[:], AF.Copy)
            nc.sync.dma_start(OUT[:, 128:256], cb[:])
            nc.sync.dma_start(OUT[:, 0:128], nb[:])

    nc.compile()
    return nc


def _get_nc():
    if "nc" not in _cache:
        _cache["nc"] = _build_nc()
    return _cache["nc"]


def make_in_maps(x, rot, noise):
    in_maps = []
    for c in range(_NCORES):
        b, h = c // 2, c % 2
        pkd = np.empty((128, 56), np.float32)
        pkd[:, :48] = np.ascontiguousarray(x[b]).reshape(128, 48)
        pkd[:, 48:51] = rot[b, 0]
        pkd[:, 51:54] = rot[b, 1]
        pkd[:, 54] = W / 2.0 * SQ_S
        pkd[:, 55] = (H / 2.0 - 64.0 * h) * SQ_S
        in_maps.append({
            "packed": pkd,
            "noise": np.ascontiguousarray(noise[b, 64 * h:64 * h + 64, :]),
        })
    return in_maps


def assemble(results):
    noisy = np.empty((B, H, W), np.float32)
    clean = np.empty((B, H, W), np.float32)
    for c in range(_NCORES):
        b, h = c // 2, c % 2
        out = results[c]["out"]
        noisy[b, 64 * h:64 * h + 64, :] = out[:, :128]
        clean[b, 64 * h:64 * h + 64, :] = out[:, 128:]
    return noisy, clean


def kernel(x, rot_matrices, noise):
    from concourse.bass_utils import run_bass_kernel_spmd

    x = np.asarray(x, dtype=np.float32)
    rot = np.asarray(rot_matrices, dtype=np.float32)
    noise = np.asarray(noise, dtype=np.float32)

    nc = _get_nc()
    res = run_bass_kernel_spmd(nc, make_in_maps(x, rot, noise),
                               list(range(_NCORES)))
    noisy, clean = assemble(res.results)
    return noisy, rot, clean


# revision 14
# speedup vs baseline: 1.0292x; 1.0027x over previous
"""CryoProjector Trainium2 kernel.

Math: clean[b,i,j] = sum_n exp(-((i-py_n)^2 + (j-px_n)^2) / (2*sigma^2))
The Gaussian is separable, so with
    Gy[n,i] = exp(-(i-py_n)^2 / (2 s^2)),  Gx[n,j] = exp(-(j-px_n)^2 / (2 s^2))
clean[b] = Gy^T @ Gx  -- a (H x N) @ (N x W) matmul done on the TensorEngine.

Sharding: 8 cores = (batch b in 0..3) x (row-half h in 0..1). Each core
computes a (64,128) slice of clean/noisy for its batch. Row offset 64h is
folded into py so the device program is identical across cores (pure SPMD).

Point->partition layout: x[b] is DMA'd contiguously as (128, 48); partition p
holds points 16p..16p+15 (3 coords interleaved). Matmul K-chunk t covers
points {16p + t : p in 0..127}, so the per-chunk per-partition scalar
px/py[:, t] feeds broadcast-AP tensor ops directly -- no transpose needed.
The chunk permutation cancels in the matmul sum.

Engine split (HW-measured rates: DVE tt 1.19ns/el fp32, Pool tt 2.7ns/el,
ACT 1.1ns/el): DVE does px/py + y-side + first x-half; Pool does the grid
iota + second x-half; ACT does the exps (fp16 out, feeding fp16 matmuls).
PE gets warm-up matmuls staged through the pipeline so its DVFS state is
high before the 16 real accumulating matmuls.

The stock bass preamble (4 const memsets + all-engine barrier) is stripped
post-build: it serializes ~3us of startup and nothing in this kernel reads
the const pool (activation bias is passed as an explicit zero tile).
"""

import numpy as np

H = W = 128
B, N = 4, 2048
SCALE = min(H, W) / 2.0 * 0.8  # 51.2
ATOM_SIGMA = 1.5
INV2SIG2 = 1.0 / (2.0 * ATOM_SIGMA**2)
SIGMA_NOISE = 0.1

_NCORES = 8
_CHUNKS = 16  # N / 128

_cache = {}


def _strip_preamble(nc):
    """Remove const-pool memsets and the startup all-engine barrier from the
    entry block. Nothing in this kernel reads the const pool, and all real
    dependencies are tracked by Tile semaphores."""
    bb = nc.m.functions[0].blocks[0]
    drop = ("InstMemset", "InstDrain", "InstEventSemaphore")
    keep = [i for i in bb.instructions if type(i).__name__ not in drop]
    removed = len(bb.instructions) - len(keep)
    try:
        bb.instructions[:] = keep
    except TypeError:
        bb.instructions.clear()
        for i in keep:
            bb.instructions.append(i)
    assert removed == 15, f"preamble shape changed: removed {removed}"


def _build_nc():
    import concourse.bacc as bacc
    import concourse.mybir as mybir
    from concourse import tile

    f32 = mybir.dt.float32
    f16 = mybir.dt.float16
    AF = mybir.ActivationFunctionType
    OP = mybir.AluOpType
    AX = mybir.AxisListType

    nc = bacc.Bacc("TRN2", target_bir_lowering=False, debug=False,
                   num_devices=_NCORES)

    # packed input: cols 0:48 = x[b] (128,48); 48:51 rot row0; 51:54 rot
    # row1; 54 cx; 55 cy_eff = 64 - 64h
    PK = nc.declare_dram_parameter("packed", (128, 56), f32, isOutput=False)
    NZ = nc.declare_dram_parameter("noise", (64, 128), f32, isOutput=False)
    OUT = nc.declare_dram_parameter("out", (64, 256), f32, isOutput=True)

    with tile.TileContext(nc) as tc:
        with (
            tc.tile_pool(name="p", bufs=1) as P,
            tc.tile_pool(name="ps", bufs=1, space="PSUM") as PS,
        ):
            pk = P.tile([128, 56], f32)
            nc.sync.dma_start(pk[:], PK[:])
            nz = P.tile([64, 128], f32)
            nc.sync.dma_start(nz[:], NZ[:])

            # Explicit zero tile: activation bias source (replaces the
            # stripped const pool) + warm-up activation input.
            w0 = P.tile([128, 1], f32)
            nc.vector.memset(w0[:], 0.0)
            w1 = P.tile([128, 1], f32)
            nc.scalar.activation(w1[:], w0[:], AF.Exp, bias=w0[:])

            # Pixel grid 0..127 directly in fp32 (exact), on Pool.
            g = P.tile([128, 128], f32)
            nc.gpsimd.iota(g[:], pattern=[[1, 128]], base=0,
                           channel_multiplier=0,
                           allow_small_or_imprecise_dtypes=True)

            # PE warm-up fodder: fp16 zero tile + separate psum bank.
            wm = P.tile([128, 128], f16)
            nc.vector.memset(wm[:], 0.0)
            wps = PS.tile([64, 128], f32)

            def warm_mm(dep_ap=None, n=1):
                # rhs read creates a dep so warm matmuls are spread through
                # the pipeline instead of all firing at t=0.
                rhs = wm[:] if dep_ap is None else dep_ap
                nfree = rhs.free_size()
                for _ in range(n):
                    nc.tensor.matmul(wps[:][:, 0:nfree], wm[:, 0:64], rhs,
                                     start=True, stop=True)

            warm_mm(n=2)

            # px, py: (128,16) each; pxy[p, t, a].
            pkv = pk[:]
            xx = pkv[:, 0:48].rearrange("p (t c) -> p t c", c=3)
            x2 = xx.unsqueeze(2).broadcast_to([128, 16, 2, 3])
            rc = pkv[:, 48:54].rearrange("p (a c) -> p a c", c=3)
            r2 = rc.unsqueeze(1).broadcast_to([128, 16, 2, 3])
            prod = P.tile([128, 96], f32)
            nc.vector.tensor_tensor(
                prod[:].rearrange("p (t a c) -> p t a c", a=2, c=3),
                x2, r2, OP.mult)
            xr = P.tile([128, 32], f32)  # [p, (t, a)]
            nc.vector.tensor_reduce(
                xr[:].rearrange("p (t a) -> p t a", a=2),
                prod[:].rearrange("p (t a c) -> p t a c", a=2, c=3),
                AX.X, OP.add)
            pxy = P.tile([128, 32], f32)
            offs = pkv[:, 54:56].unsqueeze(1).broadcast_to([128, 16, 2])
            nc.vector.scalar_tensor_tensor(
                pxy[:].rearrange("p (t a) -> p t a", a=2),
                xr[:].rearrange("p (t a) -> p t a", a=2),
                SCALE, offs, OP.mult, OP.add)
            pxy3 = pxy[:].rearrange("p (t a) -> p t a", a=2)
            px = pxy3[:, :, 0]  # (128,16) stride-2 views
            py = pxy3[:, :, 1]

            warm_mm(pxy[:].bitcast(f16)[:, 0:64])

            # ---- y side on DVE: dy, dy^2 (fp32)
            dy = P.tile([128, 16 * 64], f32)
            gy = g[:][:, 0:64].unsqueeze(1).broadcast_to([128, 16, 64])
            pyb = py.unsqueeze(2).broadcast_to([128, 16, 64])
            nc.vector.tensor_tensor(
                dy[:].rearrange("p (k j) -> p k j", j=64), gy, pyb,
                OP.subtract)
            sqy = P.tile([128, 16 * 64], f32)
            nc.vector.tensor_mul(sqy[:], dy[:], dy[:])

            warm_mm(dy[:].bitcast(f16)[:, 0:128], n=2)

            # ---- x side: first half on DVE, second half on Pool
            KH = _CHUNKS // 2
            dx = P.tile([128, 16 * 128], f32)
            sqx = P.tile([128, 16 * 128], f32)
            gxb = g[:].unsqueeze(1).broadcast_to([128, KH, 128])
            dx3 = dx[:].rearrange("p (k j) -> p k j", j=128)
            pxb3 = px.unsqueeze(2).broadcast_to([128, 16, 128])
            HALF = KH * 128

            # DVE half
            nc.vector.tensor_tensor(dx3[:, 0:KH, :], gxb,
                                    pxb3[:, 0:KH, :], OP.subtract)
            nc.vector.tensor_mul(sqx[:][:, 0:HALF], dx[:][:, 0:HALF],
                                 dx[:][:, 0:HALF])
            # Pool half
            nc.gpsimd.tensor_tensor(dx3[:, KH:, :], gxb,
                                    pxb3[:, KH:, :], OP.subtract)
            nc.gpsimd.tensor_tensor(sqx[:][:, HALF:], dx[:][:, HALF:],
                                    dx[:][:, HALF:], OP.mult)

            warm_mm(sqy[:].bitcast(f16)[:, 0:128], n=2)
            warm_mm(sqx[:].bitcast(f16)[:, 0:128], n=2)

            # ---- exps on ACT (fp16 out): EY, then EXa, then EXb
            ey = P.tile([128, 16 * 64], f16)
            nc.scalar.activation(ey[:], sqy[:], AF.Exp, bias=w0[:],
                                 scale=-INV2SIG2)
            ex = P.tile([128, 16 * 128], f16)
            nc.scalar.activation(ex[:][:, 0:HALF], sqx[:][:, 0:HALF],
                                 AF.Exp, bias=w0[:], scale=-INV2SIG2)
            nc.scalar.activation(ex[:][:, HALF:], sqx[:][:, HALF:],
                                 AF.Exp, bias=w0[:], scale=-INV2SIG2)

            # ---- 16 accumulating matmuls
            pst = PS.tile([64, 128], f32)
            eyv = ey[:].rearrange("p (k j) -> p k j", j=64)
            exv = ex[:].rearrange("p (k j) -> p k j", j=128)
            for k in range(_CHUNKS):
                nc.tensor.matmul(pst[:], eyv[:, k, :], exv[:, k, :],
                                 start=(k == 0), stop=(k == _CHUNKS - 1))

            # ---- tail
            nb = P.tile([64, 128], f32)
            nc.vector.scalar_tensor_tensor(nb[:], nz[:], SIGMA_NOISE,
                                           pst[:], OP.mult, OP.add)
            cb = P.tile([64, 128], f32)
            nc.scalar.activation(cb[:], pst[:], AF.Copy)
            nc.sync.dma_start(OUT[:, 128:256], cb[:])
            nc.sync.dma_start(OUT[:, 0:128], nb[:])

    _strip_preamble(nc)
    nc.compile()
    return nc


def _get_nc():
    if "nc" not in _cache:
        _cache["nc"] = _build_nc()
    return _cache["nc"]


def make_in_maps(x, rot, noise):
    in_maps = []
    for c in range(_NCORES):
        b, h = c // 2, c % 2
        pkd = np.empty((128, 56), np.float32)
        pkd[:, :48] = np.ascontiguousarray(x[b]).reshape(128, 48)
        pkd[:, 48:51] = rot[b, 0]
        pkd[:, 51:54] = rot[b, 1]
        pkd[:, 54] = W / 2.0
        pkd[:, 55] = H / 2.0 - 64.0 * h
        in_maps.append({
            "packed": pkd,
            "noise": np.ascontiguousarray(noise[b, 64 * h:64 * h + 64, :]),
        })
    return in_maps


def assemble(results):
    noisy = np.empty((B, H, W), np.float32)
    clean = np.empty((B, H, W), np.float32)
    for c in range(_NCORES):
        b, h = c // 2, c % 2
        out = results[c]["out"]
        noisy[b, 64 * h:64 * h + 64, :] = out[:, :128]
        clean[b, 64 * h:64 * h + 64, :] = out[:, 128:]
    return noisy, clean


def kernel(x, rot_matrices, noise):
    from concourse.bass_utils import run_bass_kernel_spmd

    x = np.asarray(x, dtype=np.float32)
    rot = np.asarray(rot_matrices, dtype=np.float32)
    noise = np.asarray(noise, dtype=np.float32)

    nc = _get_nc()
    res = run_bass_kernel_spmd(nc, make_in_maps(x, rot, noise),
                               list(range(_NCORES)))
    noisy, clean = assemble(res.results)
    return noisy, rot, clean


# revision 19
# speedup vs baseline: 1.3726x; 1.3337x over previous
"""CryoProjector Trainium2 kernel.

Math: clean[b,i,j] = sum_n exp(-((i-py_n)^2 + (j-px_n)^2) / (2*sigma^2))
The Gaussian is separable, so with
    Gy[n,i] = exp(-(i-py_n)^2 / (2 s^2)),  Gx[n,j] = exp(-(j-px_n)^2 / (2 s^2))
clean[b] = Gy^T @ Gx  -- a (H x N) @ (N x W) matmul done on the TensorEngine.

Sharding: 8 cores = (batch b in 0..3) x (row-half h in 0..1). Each core
computes a (64,128) slice of clean/noisy for its batch. Row offset 64h is
folded into py so the device program is identical across cores (pure SPMD).

Point->partition layout: x[b] is DMA'd contiguously as (128, 48); partition p
holds points 16p..16p+15 (3 coords interleaved). Matmul K-chunk t covers
points {16p + t : p in 0..127}, so the per-chunk per-partition scalar
px/py[:, t] feeds broadcast-AP tensor ops directly -- no transpose needed.
The chunk permutation cancels in the matmul sum.

Pipeline: a custom DVE op (SQDIFF_ANT: out = (src0 - src1)^2, registered
per-NEFF via the dve_ops table) builds the squared distances in a single
1-elem/cycle pass from broadcast APs of the pixel grid and px/py; ACT does
the exps (fp16 out); PE runs 16 accumulating fp16 matmuls, pre-warmed with
dummy matmuls staged through the pipeline so its DVFS clock is up.

Both the stock bass preamble (const memsets + all-engine barrier) and most
of the Tile exit epilogue (two all-engine barriers + sem clears) are
stripped post-build: the walrus NEFF postamble already syncs all engines in
an exit ring and resets every semaphore; only the SP-side DMA-completion
waits are kept so outputs are in DRAM before the NEFF exit.
"""

import numpy as np

H = W = 128
B, N = 4, 2048
SCALE = min(H, W) / 2.0 * 0.8  # 51.2
ATOM_SIGMA = 1.5
INV2SIG2 = 1.0 / (2.0 * ATOM_SIGMA**2)
SIGMA_NOISE = 0.1

_NCORES = 8
_CHUNKS = 16  # N / 128

_cache = {}


def _sqdiff_op():
    """Register (once) the fused squared-difference custom DVE op."""
    from concourse import dve_ops
    from concourse.dve_spec import Spec, Src0, Src1, sq, lower
    from concourse.dve_uop import DveOpSpec

    for o in dve_ops.OPS:
        if o.name == "SQDIFF_ANT":
            return o
    spec = Spec(
        body=sq(Src0 - Src1),
        reference=lambda in0, in1, s0, s1, imm2: (in0 - in1) ** 2,
    )
    op = dve_ops.DveOp("SQDIFF_ANT", spec, subdim=False, uops_sha={})
    dve_ops.OPS.append(op)
    # refresh the module-level snapshots keyed on OPS
    dve_ops._SUB_OPCODE_FOR_NAME[op.name] = (
        dve_ops._CUSTOM_DVE_ROW_BASE + len(dve_ops.OPS) - 1)
    dve_ops.CUSTOM_DVE_SPECS[op.name] = spec
    opcode = dve_ops.get_dve_sub_opcode(op.name)
    for ver in ("v3", "v4"):
        s = DveOpSpec(name=op.name, opcode=opcode,
                      uops=lower(spec, ver=ver), rd1_en=True)
        op.uops_sha[ver] = s.sha(ver)
    return op


def _strip_preamble(nc):
    """Remove const-pool memsets and the startup all-engine barrier from the
    entry block; nothing in this kernel reads the const pool."""
    bb = nc.m.functions[0].blocks[0]
    drop = ("InstMemset", "InstDrain", "InstEventSemaphore")
    keep = [i for i in bb.instructions if type(i).__name__ not in drop]
    removed = len(bb.instructions) - len(keep)
    bb.instructions[:] = keep
    assert removed == 15, f"preamble shape changed: removed {removed}"


def _strip_epilogue(nc):
    """In the tile-exit block keep only the SP EventSemaphores that wait for
    the output DMAs; the walrus postamble's exit ring already syncs engines
    and resets all semaphores."""
    import concourse.mybir as mybir

    blocks = nc.m.functions[0].blocks
    end = None
    for bb in blocks:
        if getattr(bb, "name", "").endswith("_end"):
            end = bb
    assert end is not None, "tile end block not found"
    keep = []
    for i in end.instructions:
        if (type(i).__name__ == "InstEventSemaphore"
                and i.engine == mybir.EngineType.SP
                and i.sync_info is not None and i.sync_info.on_wait
                and any("DMAHW" in (w.ant_name or "")
                        for w in i.sync_info.on_wait)):
            keep.append(i)
    assert len(keep) >= 2, f"expected SP dma waits, got {len(keep)}"
    end.instructions[:] = keep


def _build_nc():
    import concourse.bacc as bacc
    import concourse.mybir as mybir
    from concourse import tile

    f32 = mybir.dt.float32
    f16 = mybir.dt.float16
    AF = mybir.ActivationFunctionType
    OP = mybir.AluOpType
    AX = mybir.AxisListType
    SQDIFF = _sqdiff_op()

    nc = bacc.Bacc("TRN2", target_bir_lowering=False, debug=False,
                   num_devices=_NCORES)

    # packed input: cols 0:48 = x[b] (128,48); 48:51 rot row0; 51:54 rot
    # row1; 54 cx; 55 cy_eff = 64 - 64h
    PK = nc.declare_dram_parameter("packed", (128, 56), f32, isOutput=False)
    NZ = nc.declare_dram_parameter("noise", (64, 128), f32, isOutput=False)
    OUT = nc.declare_dram_parameter("out", (64, 256), f32, isOutput=True)

    with tile.TileContext(nc) as tc:
        with (
            tc.tile_pool(name="p", bufs=1) as P,
            tc.tile_pool(name="ps", bufs=1, space="PSUM") as PS,
        ):
            pk = P.tile([128, 56], f32)
            nc.sync.dma_start(pk[:], PK[:])
            nz = P.tile([64, 128], f32)
            nc.sync.dma_start(nz[:], NZ[:])

            # Explicit zero tile: activation bias source (replaces the
            # stripped const pool) + warm-up activation input.
            w0 = P.tile([128, 1], f32)
            nc.vector.memset(w0[:], 0.0)
            w1 = P.tile([128, 1], f32)
            nc.scalar.activation(w1[:], w0[:], AF.Exp, bias=w0[:])

            # Pixel grid 0..127 directly in fp32 (exact), on Pool.
            g = P.tile([128, 128], f32)
            nc.gpsimd.iota(g[:], pattern=[[1, 128]], base=0,
                           channel_multiplier=0,
                           allow_small_or_imprecise_dtypes=True)

            # PE warm-up fodder: fp16 zero tile + separate psum bank.
            wm = P.tile([128, 128], f16)
            nc.vector.memset(wm[:], 0.0)
            wps = PS.tile([64, 128], f32)

            def warm_mm(dep_ap=None, n=1):
                # rhs read creates a dep so warm matmuls are spread through
                # the pipeline instead of all firing at t=0.
                rhs = wm[:] if dep_ap is None else dep_ap
                nfree = rhs.free_size()
                for _ in range(n):
                    nc.tensor.matmul(wps[:][:, 0:nfree], wm[:, 0:64], rhs,
                                     start=True, stop=True)

            warm_mm(n=2)

            # px, py: (128,16) each; pxy[p, t, a].
            pkv = pk[:]
            xx = pkv[:, 0:48].rearrange("p (t c) -> p t c", c=3)
            x2 = xx.unsqueeze(2).broadcast_to([128, 16, 2, 3])
            rc = pkv[:, 48:54].rearrange("p (a c) -> p a c", c=3)
            r2 = rc.unsqueeze(1).broadcast_to([128, 16, 2, 3])
            prod = P.tile([128, 96], f32)
            nc.vector.tensor_tensor(
                prod[:].rearrange("p (t a c) -> p t a c", a=2, c=3),
                x2, r2, OP.mult)
            xr = P.tile([128, 32], f32)  # [p, (t, a)]
            nc.vector.tensor_reduce(
                xr[:].rearrange("p (t a) -> p t a", a=2),
                prod[:].rearrange("p (t a c) -> p t a c", a=2, c=3),
                AX.X, OP.add)
            pxy = P.tile([128, 32], f32)
            offs = pkv[:, 54:56].unsqueeze(1).broadcast_to([128, 16, 2])
            nc.vector.scalar_tensor_tensor(
                pxy[:].rearrange("p (t a) -> p t a", a=2),
                xr[:].rearrange("p (t a) -> p t a", a=2),
                SCALE, offs, OP.mult, OP.add)
            pxy3 = pxy[:].rearrange("p (t a) -> p t a", a=2)
            px = pxy3[:, :, 0]  # (128,16) stride-2 views
            py = pxy3[:, :, 1]

            warm_mm(pxy[:].bitcast(f16)[:, 0:64])

            # ---- squared distances via the fused custom DVE op
            sqy = P.tile([128, 16 * 64], f32)
            gy = g[:][:, 0:64].unsqueeze(1).broadcast_to([128, 16, 64])
            pyb = py.unsqueeze(2).broadcast_to([128, 16, 64])
            nc.vector._custom_dve(
                SQDIFF, out=sqy[:].rearrange("p (k j) -> p k j", j=64),
                in0=gy, in1=pyb)

            warm_mm(sqy[:].bitcast(f16)[:, 0:128], n=2)

            KH = _CHUNKS // 2
            sqx = P.tile([128, 16 * 128], f32)
            sqx3 = sqx[:].rearrange("p (k j) -> p k j", j=128)
            gxb = g[:].unsqueeze(1).broadcast_to([128, KH, 128])
            pxb3 = px.unsqueeze(2).broadcast_to([128, 16, 128])
            HALF = KH * 128
            nc.vector._custom_dve(SQDIFF, out=sqx3[:, 0:KH, :], in0=gxb,
                                  in1=pxb3[:, 0:KH, :])
            warm_mm(sqx[:].bitcast(f16)[:, 0:128], n=2)
            nc.vector._custom_dve(SQDIFF, out=sqx3[:, KH:, :], in0=gxb,
                                  in1=pxb3[:, KH:, :])

            # ---- exps on ACT (fp16 out): EY, then EXa, then EXb
            ey = P.tile([128, 16 * 64], f16)
            nc.scalar.activation(ey[:], sqy[:], AF.Exp, bias=w0[:],
                                 scale=-INV2SIG2)
            ex = P.tile([128, 16 * 128], f16)
            nc.scalar.activation(ex[:][:, 0:HALF], sqx[:][:, 0:HALF],
                                 AF.Exp, bias=w0[:], scale=-INV2SIG2)
            nc.scalar.activation(ex[:][:, HALF:], sqx[:][:, HALF:],
                                 AF.Exp, bias=w0[:], scale=-INV2SIG2)

            # ---- 16 accumulating matmuls
            pst = PS.tile([64, 128], f32)
            eyv = ey[:].rearrange("p (k j) -> p k j", j=64)
            exv = ex[:].rearrange("p (k j) -> p k j", j=128)
            for k in range(_CHUNKS):
                nc.tensor.matmul(pst[:], eyv[:, k, :], exv[:, k, :],
                                 start=(k == 0), stop=(k == _CHUNKS - 1))

            # ---- tail
            nb = P.tile([64, 128], f32)
            nc.vector.scalar_tensor_tensor(nb[:], nz[:], SIGMA_NOISE,
                                           pst[:], OP.mult, OP.add)
            cb = P.tile([64, 128], f32)
            nc.scalar.activation(cb[:], pst[:], AF.Copy)
            nc.sync.dma_start(OUT[:, 128:256], cb[:])
            nc.sync.dma_start(OUT[:, 0:128], nb[:])

    _strip_preamble(nc)
    nc.compile()
    _strip_epilogue(nc)
    return nc


def _get_nc():
    if "nc" not in _cache:
        _cache["nc"] = _build_nc()
    return _cache["nc"]


def make_in_maps(x, rot, noise):
    in_maps = []
    for c in range(_NCORES):
        b, h = c // 2, c % 2
        pkd = np.empty((128, 56), np.float32)
        pkd[:, :48] = np.ascontiguousarray(x[b]).reshape(128, 48)
        pkd[:, 48:51] = rot[b, 0]
        pkd[:, 51:54] = rot[b, 1]
        pkd[:, 54] = W / 2.0
        pkd[:, 55] = H / 2.0 - 64.0 * h
        in_maps.append({
            "packed": pkd,
            "noise": np.ascontiguousarray(noise[b, 64 * h:64 * h + 64, :]),
        })
    return in_maps


def assemble(results):
    noisy = np.empty((B, H, W), np.float32)
    clean = np.empty((B, H, W), np.float32)
    for c in range(_NCORES):
        b, h = c // 2, c % 2
        out = results[c]["out"]
        noisy[b, 64 * h:64 * h + 64, :] = out[:, :128]
        clean[b, 64 * h:64 * h + 64, :] = out[:, 128:]
    return noisy, clean


def kernel(x, rot_matrices, noise):
    from concourse.bass_utils import run_bass_kernel_spmd

    x = np.asarray(x, dtype=np.float32)
    rot = np.asarray(rot_matrices, dtype=np.float32)
    noise = np.asarray(noise, dtype=np.float32)

    nc = _get_nc()
    res = run_bass_kernel_spmd(nc, make_in_maps(x, rot, noise),
                               list(range(_NCORES)))
    noisy, clean = assemble(res.results)
    return noisy, rot, clean
